# revision 1
# baseline (speedup 1.0000x reference)
"""Trainium2 Bass kernel for a custom GRU (nn_BasicGRU).

Reference computation (per batch row b, h0 = 0):
    for t in 0..T-1:
        comb  = [x_t, h]                          # [I+H]
        z     = sigmoid(comb @ Wz + bz)
        r     = sigmoid(comb @ Wr + br)
        comb2 = [x_t, r*h]
        hc    = tanh(comb2 @ Wh + bh)
        h     = (1-z)*h + z*hc
        y_t   = h

Shapes: x [128, 1024, 256] f32, W* [768, 512] f32, b* [512] f32,
y [128, 1024, 512] f32.

Strategy (8 NeuronCores, data-parallel over batch, 16 rows/core):
- All on-chip state kept "H-major": feature dim on partitions, batch in the
  free dim.  State h is [128 partitions, 4 chunks, 16 batch] (feature
  f = chunk*128 + partition).  This makes every per-step elementwise op a
  cheap [128, 64] op and avoids any transposes in the recurrence.
- Weights are the matmul stationary operand (lhsT = W[kchunk, mchunk] tile),
  the streamed operand is the small h / (r*h) tile [128, 16].
- The x-dependent part of all three gate pre-activations (x_t @ Wx_g + b_g)
  does not depend on the recurrence: it is computed by efficient batched
  matmuls directly into the same PSUM banks the recurrent matmuls then
  accumulate into (one bank per gate per 8-step block).
- Recurrent weights and gate weights are cast to bf16 (fp32 PSUM
  accumulation); the master h state stays fp32, with a bf16 copy made each
  step for the next step's matmuls.
- Output y is written H-major to HBM and rearranged to [B, T, H] on host.
"""

import numpy as np
import ml_dtypes

import concourse.bass as bass
import concourse.tile as tile
from concourse import bacc, mybir
from concourse.bass_utils import run_bass_kernel_spmd

F32 = mybir.dt.float32
BF16 = mybir.dt.bfloat16

N_CORES = 8
B = 128
I_DIM = 256
H_DIM = 512
B_LOC = B // N_CORES          # 16 batch rows per core
BLK = 8                       # recurrence steps per PSUM block
KC = H_DIM // 128             # 4 k-chunks for the h-part contraction
MC = H_DIM // 128             # 4 output-feature chunks
IC = I_DIM // 128             # 2 k-chunks for the x-part contraction
GZ, GR, GH = 0, 1, 2          # gate order in the stacked weight tensors

_CACHE = {}


def build_gru_nc(T, repeat=1, chains=1, loop_blocks=0):
    """Build the Bass/Tile program for a T-step GRU on one core.

    repeat>1 wraps the whole recurrence in a hardware For_i loop that
    re-runs it from h0 (used only for on-device timing via the R-slope).
    chains=2 splits the 16 batch rows into two independent recurrence
    chains per core so their serial latencies overlap (separate PSUM
    banks per chain; PSUM single-buffered with x-part matmuls at block
    starts).
    loop_blocks=LB>0 emits a hardware For_i loop whose body processes LB
    8-step blocks (keeps the instruction stream small: long fully
    unrolled streams measure ~25% slower per step due to instruction
    fetch).  State is carried across iterations in dedicated tiles."""
    NB = T // BLK
    assert T % BLK == 0

    nc = bacc.Bacc("TRN2", target_bir_lowering=False, debug=False,
                   enable_asserts=False, num_devices=N_CORES)

    xT_cols = T * B_LOC + (loop_blocks * BLK * B_LOC if loop_blocks else 0)
    xT = nc.dram_tensor("xT", [IC, 128, xT_cols], BF16, kind="ExternalInput")
    whh = nc.dram_tensor("whh", [128, 3, KC, MC, 128], BF16, kind="ExternalInput")
    wx = nc.dram_tensor("wx", [128, 3, IC, MC, 128], BF16, kind="ExternalInput")
    bias = nc.dram_tensor("bias", [1, 3, MC, 128], BF16, kind="ExternalInput")
    y = nc.dram_tensor("y", [128, MC, T * B_LOC], BF16, kind="ExternalOutput")

    with tile.TileContext(nc) as tc:
        with (
            tc.tile_pool(name="const", bufs=1) as const,
            tc.tile_pool(name="xp", bufs=4) as xp,
            tc.tile_pool(name="yp", bufs=3) as yp,
            tc.tile_pool(name="hp", bufs=4) as hp,
            tc.tile_pool(name="sp", bufs=4) as sp,
            tc.tile_pool(name="ps", bufs=(2 if chains == 1 else 1),
                         space="PSUM") as ps,
        ):
            # ---- constants ----
            whh_s = const.tile([128, 3, KC, MC, 128], BF16, tag="whh")
            nc.sync.dma_start(whh_s[:], whh[:])
            wx_s = const.tile([128, 3, IC, MC, 128], BF16, tag="wx")
            nc.sync.dma_start(wx_s[:], wx[:])
            bias_s = const.tile([1, 3, MC, 128], BF16, tag="bias")
            nc.sync.dma_start(bias_s[:], bias[:])
            ones_s = const.tile([1, BLK * B_LOC], BF16, tag="ones")
            nc.vector.memset(ones_s[:], 1.0)
            h0_b = const.tile([128, KC, B_LOC], BF16, tag="h0b")
            nc.vector.memset(h0_b[:], 0.0)

            def alloc_block(c0):
                """Allocate block tiles, start the x DMA (c0 = dynamic or
                static column offset), and return the per-block state;
                x-part matmuls are emitted separately so they can
                interleave with the previous block's steps."""
                w = BLK * B_LOC
                xt = xp.tile([128, IC, BLK * B_LOC], BF16, tag="xt",
                             name="xt")
                nc.sync.dma_start(xt[:],
                                  xT[:, :, bass.ds(c0, w)].rearrange(
                                      "i p n -> p i n"))
                bw = B_LOC // chains
                xtv = xt[:].rearrange("p i (t b) -> p i t b", b=B_LOC)
                psg = [[ps.tile([128, MC, BLK * bw], F32, tag=f"ps{g}c{c}",
                                name=f"ps{g}c{c}") for c in range(chains)]
                       for g in range(3)]
                # x-part matmul work list (per chain)
                work = []
                for g in range(3):
                    for mc in range(MC):
                        for c in range(chains):
                            out_ap = psg[g][c][:, mc, :]
                            rhs0 = xtv[:, 0, :, c * bw:(c + 1) * bw]
                            rhs1 = xtv[:, 1, :, c * bw:(c + 1) * bw]
                            work.append((out_ap, wx_s[:, g, 0, mc, :], rhs0,
                                         mc == 0))
                            work.append((out_ap, wx_s[:, g, 1, mc, :], rhs1,
                                         False))
                            work.append((out_ap, bias_s[:, g, mc, :],
                                         ones_s[:, :BLK * bw], False))
                return psg, work

            def emit_xpre(work, n):
                for _ in range(min(n, len(work))):
                    out_ap, lhsT, rhs, is_start = work.pop(0)
                    nc.tensor.matmul(out_ap, lhsT, rhs,
                                     start=is_start, stop=False)

            def emit_body():
                bw = B_LOC // chains
                h_prev_b = [h0_b[:, :, c * bw:(c + 1) * bw]
                            for c in range(chains)]
                h_prev_f = h_prev_b
                psg, xwork = alloc_block(0)
                emit_xpre(xwork, len(xwork))

                for blk in range(NB):
                    c0 = blk * BLK * B_LOC
                    c1 = (blk + 1) * BLK * B_LOC
                    ys = yp.tile([128, MC, BLK * B_LOC], BF16, tag="ys")
                    ysv = ys.rearrange("p m (t b) -> p m t b", b=B_LOC)
                    if blk + 1 < NB:
                        psg_next, xwork = alloc_block((blk + 1) * BLK * B_LOC)
                    else:
                        psg_next, xwork = None, []
                    if chains > 1:
                        # single-buffered PSUM: next block's x-parts go in
                        # one burst at the block boundary
                        emit_xpre(xwork, len(xwork))

                    for tl in range(BLK):
                        s0 = tl * bw
                        s1 = (tl + 1) * bw
                        # r then z gate: accumulate h @ Whh_g onto the x-part
                        for g in (GR, GZ):
                            for c in range(chains):
                                for mc in range(MC):
                                    for kc in range(KC):
                                        nc.tensor.matmul(
                                            psg[g][c][:, mc, s0:s1],
                                            whh_s[:, g, kc, mc, :],
                                            h_prev_b[c][:, kc, :],
                                            start=False, stop=(kc == KC - 1),
                                        )
                        r_b, rh_b, z_b, negb_f = [], [], [], []
                        for c in range(chains):
                            rb = sp.tile([128, MC, bw], BF16, tag=f"r_b{c}",
                                         name=f"r_b{c}")
                            nc.scalar.activation(
                                rb[:], psg[GR][c][:, :, s0:s1],
                                func=mybir.ActivationFunctionType.Sigmoid)
                            r_b.append(rb)
                        for c in range(chains):
                            rhb = sp.tile([128, MC, bw], BF16, tag=f"rh_b{c}",
                                          name=f"rh_b{c}")
                            nc.vector.tensor_mul(rhb[:], r_b[c][:],
                                                 h_prev_b[c])
                            rh_b.append(rhb)
                        for c in range(chains):
                            zb = sp.tile([128, MC, bw], BF16, tag=f"z_b{c}",
                                         name=f"z_b{c}")
                            nc.scalar.activation(
                                zb[:], psg[GZ][c][:, :, s0:s1],
                                func=mybir.ActivationFunctionType.Sigmoid)
                            z_b.append(zb)
                        for c in range(chains):
                            nb_ = sp.tile([128, MC, bw], F32, tag=f"negb{c}",
                                          name=f"negb{c}")
                            nc.vector.scalar_tensor_tensor(
                                nb_[:], z_b[c][:], 1.0, h_prev_b[c],
                                op0=mybir.AluOpType.subtract,
                                op1=mybir.AluOpType.mult)
                            negb_f.append(nb_)

                        # candidate gate: (r*h) @ Whh_h
                        for c in range(chains):
                            for mc in range(MC):
                                for kc in range(KC):
                                    nc.tensor.matmul(
                                        psg[GH][c][:, mc, s0:s1],
                                        whh_s[:, GH, kc, mc, :],
                                        rh_b[c][:, kc, :],
                                        start=False, stop=(kc == KC - 1),
                                    )
                        if chains == 1:
                            # next block's x-part matmuls fill the PE idle
                            # gap while the tanh/blend tail runs
                            emit_xpre(xwork, 5)
                        hc_f = []
                        for c in range(chains):
                            hcf = sp.tile([128, MC, bw], F32, tag=f"hc_f{c}",
                                          name=f"hc_f{c}")
                            nc.scalar.activation(
                                hcf[:], psg[GH][c][:, :, s0:s1],
                                func=mybir.ActivationFunctionType.Tanh)
                            hc_f.append(hcf)

                        # blend h' = z*hc - (z-1)*h: on-path a = z*hc then
                        # h'_bf16 = a - negb (bf16 out feeds next step's
                        # matmuls); fp32 master (y staging) off-path.
                        new_b = []
                        for c in range(chains):
                            af = sp.tile([128, MC, bw], F32, tag=f"a_f{c}",
                                         name=f"a_f{c}")
                            nc.vector.tensor_mul(af[:], z_b[c][:], hc_f[c][:])
                            hnb = ysv[:, :, tl, c * bw:(c + 1) * bw]
                            nc.vector.tensor_sub(hnb, af[:], negb_f[c][:])
                            new_b.append(hnb)

                        h_prev_b = new_b

                    nc.gpsimd.dma_start(y[:, :, bass.ds(c0, BLK * B_LOC)], ys[:])
                    psg = psg_next

            def emit_loop():
                assert chains == 1
                LB = loop_blocks
                assert NB % LB == 0 and LB % 2 == 0
                n_iters = NB // LB
                bw = BLK * B_LOC

                carry_b = const.tile([128, KC, B_LOC], BF16, tag="carryb")
                nc.vector.memset(carry_b[:], 0.0)
                # fixed double-buffered cross-edge tiles (explicit parity so
                # addresses line up across the loop back-edge)
                xt_bufs = [const.tile([128, IC, bw], BF16, tag=f"xtb{i}",
                                      name=f"xtb{i}") for i in range(2)]
                ps_bufs = [[ps.tile([128, MC, bw], F32, tag=f"ps{g}b{i}",
                                    name=f"ps{g}b{i}", bufs=1)
                            for i in range(2)] for g in range(3)]

                def alloc_block_fixed(c0, parity):
                    xt = xt_bufs[parity]
                    nc.sync.dma_start(
                        xt[:], xT[:, :, bass.ds(c0, bw)].rearrange(
                            "i p n -> p i n"))
                    psg = [ps_bufs[g][parity] for g in range(3)]
                    work = []
                    for g in range(3):
                        for mc in range(MC):
                            out_ap = psg[g][:, mc, :]
                            work.append((out_ap, wx_s[:, g, 0, mc, :],
                                         xt[:, 0, :], mc == 0))
                            work.append((out_ap, wx_s[:, g, 1, mc, :],
                                         xt[:, 1, :], False))
                            work.append((out_ap, bias_s[:, g, mc, :],
                                         ones_s[:], False))
                    return psg, work

                psg, xwork = alloc_block_fixed(0, 0)
                emit_xpre(xwork, len(xwork))

                with tc.For_i(0, n_iters, 1) as iv:
                    base = iv * (LB * bw)
                    h_prev_b = carry_b[:]
                    for bi in range(LB):
                        c0 = base + bi * bw
                        ys = yp.tile([128, MC, bw], BF16, tag="ys", name="ys")
                        ysv = ys.rearrange("p m (t b) -> p m t b", b=B_LOC)
                        psg_next, xwork = alloc_block_fixed(
                            base + (bi + 1) * bw, (bi + 1) % 2)

                        for tl in range(BLK):
                            s0 = tl * B_LOC
                            s1 = (tl + 1) * B_LOC
                            last = (bi == LB - 1 and tl == BLK - 1)
                            # r-gate matmuls only; the z-gate ones are
                            # deferred past the hc matmuls so the z chain
                            # hides inside the hc->tanh window
                            for mc in range(MC):
                                for kc in range(KC):
                                    nc.tensor.matmul(
                                        psg[GR][:, mc, s0:s1],
                                        whh_s[:, GR, kc, mc, :],
                                        h_prev_b[:, kc, :],
                                        start=False, stop=(kc == KC - 1),
                                    )
                            r_b = sp.tile([128, MC, B_LOC], BF16, tag="r_b",
                                          name="r_b")
                            nc.scalar.activation(
                                r_b[:], psg[GR][:, :, s0:s1],
                                func=mybir.ActivationFunctionType.Sigmoid)
                            rh_b = sp.tile([128, MC, B_LOC], BF16,
                                           tag="rh_b", name="rh_b")
                            nc.vector.tensor_mul(rh_b[:], r_b[:], h_prev_b)
                            emit_xpre(xwork, 5)
                            for mc in range(MC):
                                for kc in range(KC):
                                    nc.tensor.matmul(
                                        psg[GH][:, mc, s0:s1],
                                        whh_s[:, GH, kc, mc, :],
                                        rh_b[:, kc, :],
                                        start=False, stop=(kc == KC - 1),
                                    )
                            for mc in range(MC):
                                for kc in range(KC):
                                    nc.tensor.matmul(
                                        psg[GZ][:, mc, s0:s1],
                                        whh_s[:, GZ, kc, mc, :],
                                        h_prev_b[:, kc, :],
                                        start=False, stop=(kc == KC - 1),
                                    )
                            hc_f = sp.tile([128, MC, B_LOC], F32,
                                           tag="hc_f", name="hc_f")
                            nc.scalar.activation(
                                hc_f[:], psg[GH][:, :, s0:s1],
                                func=mybir.ActivationFunctionType.Tanh)
                            z_b = sp.tile([128, MC, B_LOC], BF16, tag="z_b",
                                          name="z_b")
                            nc.scalar.activation(
                                z_b[:], psg[GZ][:, :, s0:s1],
                                func=mybir.ActivationFunctionType.Sigmoid)
                            negb_f = sp.tile([128, MC, B_LOC], F32,
                                             tag="negb_f", name="negb_f")
                            nc.vector.scalar_tensor_tensor(
                                negb_f[:], z_b[:], 1.0, h_prev_b,
                                op0=mybir.AluOpType.subtract,
                                op1=mybir.AluOpType.mult)
                            a_f = sp.tile([128, MC, B_LOC], F32, tag="a_f",
                                          name="a_f")
                            nc.vector.tensor_mul(a_f[:], z_b[:], hc_f[:])
                            h_new_b = ysv[:, :, tl, :]
                            nc.vector.tensor_sub(h_new_b, a_f[:], negb_f[:])
                            if last:
                                nc.vector.tensor_sub(carry_b[:], a_f[:],
                                                     negb_f[:])
                            h_prev_b = carry_b[:] if last else h_new_b

                        nc.gpsimd.dma_start(
                            y[:, :, bass.ds(c0, bw)], ys[:])
                        psg = psg_next

            if loop_blocks:
                if repeat == 1:
                    emit_loop()
                else:
                    with tc.For_i(0, repeat, 1):
                        emit_loop()
            elif repeat == 1:
                emit_body()
            else:
                with tc.For_i(0, repeat, 1):
                    emit_body()

    nc.finalize()
    return nc


def _host_prep_weights(Wz, bz, Wr, br, Wh, bh):
    Wst = np.stack([Wz, Wr, Wh])                     # [3, 768, 512]
    wx_host = np.ascontiguousarray(
        Wst[:, :I_DIM, :].reshape(3, IC, 128, MC, 128).transpose(2, 0, 1, 3, 4)
    ).astype(ml_dtypes.bfloat16)                     # [128, 3, IC, MC, 128]
    whh_host = np.ascontiguousarray(
        Wst[:, I_DIM:, :].reshape(3, KC, 128, MC, 128).transpose(2, 0, 1, 3, 4)
    ).astype(ml_dtypes.bfloat16)                     # [128, 3, KC, MC, 128]
    bias_host = np.stack([bz, br, bh]).reshape(1, 3, MC, 128).astype(
        ml_dtypes.bfloat16)
    return wx_host, whh_host, bias_host


def make_in_maps(x, Wz, bz, Wr, br, Wh, bh, pad_blocks=0):
    x = np.asarray(x)
    T = x.shape[1]
    assert x.shape == (B, T, I_DIM)
    wx_host, whh_host, bias_host = _host_prep_weights(
        np.asarray(Wz), np.asarray(bz), np.asarray(Wr), np.asarray(br),
        np.asarray(Wh), np.asarray(bh))
    in_maps = []
    for c in range(N_CORES):
        xc = x[c * B_LOC:(c + 1) * B_LOC]            # [16, T, 256]
        xTc = np.ascontiguousarray(xc.transpose(2, 1, 0)).reshape(
            IC, 128, T * B_LOC).astype(ml_dtypes.bfloat16)
        if pad_blocks:
            pad = np.zeros((IC, 128, pad_blocks * BLK * B_LOC),
                           ml_dtypes.bfloat16)
            xTc = np.concatenate([xTc, pad], axis=2)
        in_maps.append({
            "xT": xTc,
            "whh": whh_host,
            "wx": wx_host,
            "bias": bias_host,
        })
    return in_maps


def assemble_output(y_cat, T):
    """y_cat: [N_CORES*128, MC, T*B_LOC] (concatenated per-core 'y' outputs)
    -> [B, T, H]."""
    y_cat = np.asarray(y_cat).reshape(N_CORES, 128, MC, T, B_LOC)
    out = y_cat.transpose(0, 4, 3, 2, 1).reshape(B, T, H_DIM)
    return np.ascontiguousarray(out, dtype=np.float32)


def kernel(x, Wz, bz, Wr, br, Wh, bh):
    x = np.asarray(x)
    T = x.shape[1]
    # hardware For_i variant when the block structure allows it (identical
    # numerics, far faster to compile); fully unrolled fallback otherwise
    NBt = T // BLK
    lb = next((n for n in (32, 16, 8) if NBt % n == 0), 0)
    in_maps = make_in_maps(x, Wz, bz, Wr, br, Wh, bh, pad_blocks=lb)

    key = (T, lb)
    if key not in _CACHE:
        _CACHE[key] = build_gru_nc(T, loop_blocks=lb)
    nc = _CACHE[key]

    res = run_bass_kernel_spmd(nc, in_maps, core_ids=list(range(N_CORES)))
    y_cat = np.concatenate([res.results[c]["y"] for c in range(N_CORES)], axis=0)
    return assemble_output(y_cat, T)

